# revision 5
# baseline (speedup 1.0000x reference)
"""GAT layer (nn_GATLayer) on 8 Trainium2 NeuronCores — sort-split kernel.

Math per batch (h = x@W, s1 = h@a1, s2 = h@a2, e1 = e^{0.8 s1}, e2 = e^{s2},
f2 = e^{0.2 s2}; softmax columns rescaled by e^{-0.2 s1_i}):
    p[j,i] = m[j,i] * max(e1_i * e2_j, f2_j)
    numT   = [h|1]^T @ p,  out_i = num_i / den_i

Device identity:  max(a,b) = a + relu(b-a), so
    p = e1_i * (m * e2_j)  +  m * relu(f2'_j - e1_i) * e2_j ,  f2' = f2/e2.
The relu term vanishes wherever s1_i + s2_j >= 0. Sorting j by s2 and i by
s1 (host permutations, undone on output) makes that sign boundary a
monotone staircase, so per j-tile t the i-columns split into
    [ pure e1*e2 region | boundary band | pure f2 region ] .
Pure regions are MASK-ONLY matmuls (fp8 mask moving; e2- and f2-scaled
stationaries hE/hF); only the bands (~1.6x N columns total) need
elementwise work: one ACT relu (scale/bias per partition) + one DVE
multiply per tile. The dense formulation's N^2 DVE work drops ~10x.

PSUM: G (e1-scaled accumulator) in banks 0-3, R (unit) in banks 4-7.
G-bank write regions shrink monotonically from the bank base (accumulate
semantics for that pattern validated on HW); R banks are opened full-width
by zero matmuls, after which arbitrary sub-range accumulates are valid.
num = e1*G + R and the divide run on host (O(N D) work).

Sharding: data-parallel over batch B=8 across 8 cores; mask/h are uploaded
per batch in that batch's sorted order. The program depends only on the
common (min/max over batches) band boundaries -> JIT-compiled per input,
cached by the boundary tuple.
"""

import os
import sys

sys.path.insert(0, "/opt/trn_rl_repo")

import numpy as np
import ml_dtypes

B, N, DIN, DOUT = 8, 2048, 64, 64
NCORES = 8
PJ = 128
NJT = N // PJ
BANK = 512
HC = DOUT + 2
D1 = DOUT + 1

_CACHE = {}
LAST_RESULT = None


def _build_nc(Bt, Et):
    from contextlib import ExitStack

    import concourse.tile as tile
    from concourse import bacc, mybir

    f32 = mybir.dt.float32
    bf16 = mybir.dt.bfloat16
    fp8 = mybir.dt.float8e4
    ALU = mybir.AluOpType
    AF = mybir.ActivationFunctionType

    nc = bacc.Bacc("TRN2", target_bir_lowering=False, debug=False,
                   num_devices=NCORES)

    maskp = nc.dram_tensor("maskp", [PJ, NJT * N], fp8, kind="ExternalInput").ap()
    AUXW = 2 * NJT * HC
    aux = nc.dram_tensor("aux", [PJ, AUXW], bf16, kind="ExternalInput").ap()
    e1d = nc.dram_tensor("e1d", [PJ, N], fp8, kind="ExternalInput").ap()
    # tiny head: stationaries for the two first-needed tiles (t=15,14)
    auxh = nc.dram_tensor("auxh", [PJ, 2 * HC], bf16, kind="ExternalInput").ap()
    f2p = nc.dram_tensor("f2p", [PJ, NJT], f32, kind="ExternalInput").ap()
    out = nc.dram_tensor("out", [D1, 2 * N], bf16, kind="ExternalOutput").ap()

    g_first = {}
    g_last = {}
    for t in range(NJT - 1, -1, -1):
        for k in range((int(Bt[t]) + BANK - 1) // BANK):
            if k not in g_first:
                g_first[k] = t
            g_last[k] = t

    with ExitStack() as ctx:
        tc = ctx.enter_context(tile.TileContext(nc))
        const = ctx.enter_context(tc.tile_pool(name="const", bufs=1))

        # issue order: keep the early in-flight set minimal -- DMA engines
        # share packet bandwidth fairly across ALL outstanding transfers,
        # so first-needed data must not race the mask bulk
        auxH_sb = const.tile([PJ, 2 * HC], bf16, tag="auxH")
        nc.sync.dma_start(auxH_sb[:], auxh)
        auxA_sb = const.tile([PJ, NJT * HC], bf16, tag="auxA")
        nc.sync.dma_start(auxA_sb[:], aux[:, : NJT * HC])
        f2p_sb = const.tile([PJ, NJT], f32, tag="f2p")
        auxC_sb = const.tile([PJ, N], fp8, tag="auxC")
        nc.sync.dma_start(auxC_sb[:], e1d)
        auxD_sb = const.tile([PJ, NJT * HC], bf16, tag="auxD")

        def stat_he(t):
            if t >= NJT - 2:  # head tile: [t14 | t15]
                o = (t - (NJT - 2)) * HC
                return auxH_sb[:, o : o + D1]
            return auxA_sb[:, t * HC : t * HC + D1]

        def stat_hf(t):
            return auxD_sb[:, t * HC : t * HC + D1]


        GROUPS = [(15, 16), (13, 15), (10, 13), (7, 10), (4, 7), (0, 4)]
        mpool = ctx.enter_context(tc.tile_pool(name="mask", bufs=len(GROUPS)))
        mask_grp = {}
        for gi, (lo, hi) in enumerate(GROUPS):
            mg = mpool.tile([PJ, (hi - lo) * N], fp8, tag="mb")
            nc.sync.dma_start(mg[:], maskp[:, lo * N : hi * N])
            for t in range(lo, hi):
                mask_grp[t] = (mg, t - lo)
            if gi == 0:
                nc.sync.dma_start(f2p_sb[:], f2p)
            if gi == 2:
                nc.sync.dma_start(auxD_sb[:], aux[:, NJT * HC : 2 * NJT * HC])

        def msl(t, c0, c1):
            mg, k = mask_grp[t]
            return mg[:, k * N + c0 : k * N + c1]

        pp = ctx.enter_context(tc.tile_pool(name="psum", bufs=1, space="PSUM"))
        ps = pp.tile([D1, 2 * N], f32, tag="gr")

        rpool = ctx.enter_context(tc.tile_pool(name="relu", bufs=3))
        cpool = ctx.enter_context(tc.tile_pool(name="corr", bufs=3))
        dpool = ctx.enter_context(tc.tile_pool(name="drain", bufs=2))

        # zero the R psum half on DVE during the DMA prefix; every R
        # matmul then accumulates with start=False
        nc.vector.memset(ps[:, N : 2 * N], 0.0)

        gsb = dpool.tile([D1, N], bf16, tag="gsb")
        g_drained = set()

        # ---- pass 1 (descending t): G matmuls + band corrections ----
        for t in range(NJT - 1, -1, -1):
            b0, e0 = int(Bt[t]), int(Et[t])
            w = e0 - b0
            if w > 0:
                # full p on the band: q = max(e1*e2, f2); cm = m*q -> R
                # (stationary: raw hcat); G covers only the pure [0, b0)
                r_sb = rpool.tile([PJ, w], bf16, tag="r")
                nc.vector.tensor_scalar(
                    r_sb[:], auxC_sb[:, b0:e0],
                    f2p_sb[:, t : t + 1], None,
                    op0=ALU.max,
                )
                cm_sb = cpool.tile([PJ, w], bf16, tag="cm")
                nc.vector.tensor_tensor(
                    cm_sb[:], msl(t, b0, e0), r_sb[:], op=ALU.mult
                )
            c = 0
            while c < b0:
                c1 = min(c + BANK, b0)
                k = c // BANK
                nc.tensor.matmul(
                    ps[:, c:c1], stat_he(t), msl(t, c, c1),
                    start=(g_first[k] == t), stop=(g_last[k] == t),
                    skip_group_check=True,
                )
                c = c1
            if w > 0:
                c = b0
                while c < e0:
                    c1 = min((c // BANK + 1) * BANK, e0)
                    nc.tensor.matmul(
                        ps[:, N + c : N + c1], stat_he(t),
                        cm_sb[:, c - b0 : c1 - b0],
                        start=False, stop=False, skip_group_check=True,
                    )
                    c = c1
            for k in range(4):
                if g_last.get(k) == t:
                    sl = slice(k * BANK, (k + 1) * BANK)
                    if k % 2 == 0:
                        nc.scalar.copy(gsb[:, sl], ps[:, sl])
                    else:
                        nc.vector.tensor_copy(gsb[:, sl], ps[:, sl])
                    g_drained.add(k)

        for k in range(4):
            if k not in g_drained:
                nc.vector.memset(gsb[:, k * BANK : (k + 1) * BANK], 0.0)
        nc.sync.dma_start(out[:, :N], gsb[:])

        # ---- pass 2 (ascending t): F matmuls, drain R banks inline ----
        rsb = dpool.tile([D1, N], bf16, tag="rsb")
        for t in range(NJT):
            e0 = int(Et[t])
            c = e0
            while c < N:
                c1 = min((c // BANK + 1) * BANK, N)
                last = (t == NJT - 1) or (int(Et[t + 1]) >= c1)
                nc.tensor.matmul(
                    ps[:, N + c : N + c1], stat_hf(t), msl(t, c, c1),
                    start=False, stop=last, skip_group_check=True,
                )
                if last:
                    k = c // BANK
                    sl = slice(k * BANK, (k + 1) * BANK)
                    if k % 2 == 0:
                        nc.scalar.copy(rsb[:, sl], ps[:, N + k * BANK : N + (k + 1) * BANK])
                    else:
                        nc.vector.tensor_copy(rsb[:, sl], ps[:, N + k * BANK : N + (k + 1) * BANK])
                c = c1
        nc.sync.dma_start(out[:, N:], rsb[:])

    nc.compile()
    return nc


def _prep(x, adj, W, a):
    bf = ml_dtypes.bfloat16
    f8 = ml_dtypes.float8_e4m3fn
    x = np.asarray(x, dtype=np.float32)
    W = np.ascontiguousarray(np.asarray(W, dtype=np.float32))
    a = np.asarray(a, dtype=np.float32)

    h = x @ W
    s1 = h @ a[:DOUT]
    s2 = h @ a[DOUT:]

    pj = [np.argsort(s2[b], kind="stable") for b in range(B)]
    pi = [np.argsort(-s1[b], kind="stable") for b in range(B)]

    t_arr = np.zeros((B, N), np.int64)
    for b in range(B):
        s2s = s2[b][pj[b]]
        t_arr[b] = np.searchsorted(s2s, -s1[b][pi[b]], side="left") // PJ

    Bt = np.zeros(NJT, np.int64)
    Et = np.zeros(NJT, np.int64)
    for t in range(NJT):
        Bt[t] = min(np.searchsorted(t_arr[b], t, side="left") for b in range(B))
        Et[t] = max(np.searchsorted(t_arr[b], t, side="right") for b in range(B))

    adjb = np.asarray(adj) > 0
    in_maps = []
    for b in range(B):
        m = adjb.T[np.ix_(pj[b], pi[b])]
        maskp = np.ascontiguousarray(
            m.reshape(NJT, PJ, N).transpose(1, 0, 2).reshape(PJ, NJT * N)
        ).astype(f8)

        hs = h[b][pj[b]].astype(np.float32)
        e2 = np.exp(s2[b][pj[b]]).astype(np.float32)
        f2 = np.exp(0.2 * s2[b][pj[b]]).astype(np.float32)
        f2pv = np.exp(-0.8 * s2[b][pj[b]]).astype(np.float32)

        he = np.zeros((N, HC), dtype=bf)
        he[:, :DOUT] = (hs * e2[:, None]).astype(bf)
        he[:, DOUT] = e2.astype(bf)
        hf = np.zeros((N, HC), dtype=bf)
        hf[:, :DOUT] = (hs * f2[:, None]).astype(bf)
        hf[:, DOUT] = f2.astype(bf)

        def tile_p(arr):
            return arr.reshape(NJT, PJ, HC).transpose(1, 0, 2).reshape(PJ, NJT * HC)

        e1 = np.exp(0.8 * s1[b][pi[b]]).astype(f8)
        e1d = np.ascontiguousarray(np.broadcast_to(e1[None, :], (PJ, N)))
        aux = np.concatenate([tile_p(he), tile_p(hf)], axis=1)
        f2p_t = np.ascontiguousarray(f2pv.reshape(NJT, PJ).T)

        auxh = np.ascontiguousarray(
            aux[:, (NJT - 2) * HC : NJT * HC])
        in_maps.append({
            "maskp": maskp, "aux": np.ascontiguousarray(aux),
            "f2p": f2p_t, "e1d": e1d, "auxh": auxh,
        })
    return in_maps, (tuple(Bt), tuple(Et)), (pi, s1)


def kernel(x, adj, W, a):
    global LAST_RESULT
    from concourse import bass_utils

    x = np.asarray(x)
    adj = np.asarray(adj)
    assert x.shape == (B, N, DIN) and adj.shape == (N, N)

    in_maps, key, (pi, s1) = _prep(x, adj, W, a)
    if key not in _CACHE:
        _CACHE[key] = _build_nc(np.array(key[0]), np.array(key[1]))
    nc = _CACHE[key]

    res = bass_utils.run_bass_kernel_spmd(
        nc, in_maps, core_ids=list(range(NCORES)),
        trace=bool(int(os.environ.get("GAT_TRACE", "0"))),
    )
    LAST_RESULT = res

    Bt15 = key[0][NJT - 1]
    out = np.empty((B, N, DOUT), dtype=np.float32)
    for b in range(B):
        gr = res.results[b]["out"].astype(np.float64)
        G, R = gr[:, :N], gr[:, N:]
        e1 = np.exp(0.8 * s1[b][pi[b]]).astype(np.float64)
        valid = np.arange(N) < Bt15
        num = np.where(valid[None, :], e1[None, :] * G[:DOUT], 0.0) + R[:DOUT]
        den = np.where(valid, e1 * G[DOUT], 0.0) + R[DOUT]
        outs = (num / den[None, :]).T
        out[b][pi[b]] = outs.astype(np.float32)
    return out


# revision 6
# speedup vs baseline: 1.0192x; 1.0192x over previous
"""GAT layer (nn_GATLayer) on 8 Trainium2 NeuronCores — sort-split kernel.

Math per batch (h = x@W, s1 = h@a1, s2 = h@a2, e1 = e^{0.8 s1}, e2 = e^{s2},
f2 = e^{0.2 s2}; softmax columns rescaled by e^{-0.2 s1_i}):
    p[j,i] = m[j,i] * max(e1_i * e2_j, f2_j)
    numT   = [h|1]^T @ p,  out_i = num_i / den_i

Device identity:  max(a,b) = a + relu(b-a), so
    p = e1_i * (m * e2_j)  +  m * relu(f2'_j - e1_i) * e2_j ,  f2' = f2/e2.
The relu term vanishes wherever s1_i + s2_j >= 0. Sorting j by s2 and i by
s1 (host permutations, undone on output) makes that sign boundary a
monotone staircase, so per j-tile t the i-columns split into
    [ pure e1*e2 region | boundary band | pure f2 region ] .
Pure regions are MASK-ONLY matmuls (fp8 mask moving; e2- and f2-scaled
stationaries hE/hF); only the bands (~1.6x N columns total) need
elementwise work: one ACT relu (scale/bias per partition) + one DVE
multiply per tile. The dense formulation's N^2 DVE work drops ~10x.

PSUM: G (e1-scaled accumulator) in banks 0-3, R (unit) in banks 4-7.
G-bank write regions shrink monotonically from the bank base (accumulate
semantics for that pattern validated on HW); R banks are opened full-width
by zero matmuls, after which arbitrary sub-range accumulates are valid.
num = e1*G + R and the divide run on host (O(N D) work).

Sharding: data-parallel over batch B=8 across 8 cores; mask/h are uploaded
per batch in that batch's sorted order. The program depends only on the
common (min/max over batches) band boundaries -> JIT-compiled per input,
cached by the boundary tuple.
"""

import os
import sys

sys.path.insert(0, "/opt/trn_rl_repo")

import numpy as np
import ml_dtypes

B, N, DIN, DOUT = 8, 2048, 64, 64
NCORES = 8
PJ = 128
NJT = N // PJ
BANK = 512
HC = DOUT + 2
D1 = DOUT + 1

_CACHE = {}
LAST_RESULT = None


def _build_nc(Bt, Et):
    from contextlib import ExitStack

    import concourse.tile as tile
    from concourse import bacc, mybir

    f32 = mybir.dt.float32
    bf16 = mybir.dt.bfloat16
    fp8 = mybir.dt.float8e4
    ALU = mybir.AluOpType
    AF = mybir.ActivationFunctionType

    nc = bacc.Bacc("TRN2", target_bir_lowering=False, debug=False,
                   num_devices=NCORES)

    maskp = nc.dram_tensor("maskp", [PJ, NJT * N], fp8, kind="ExternalInput").ap()
    AUXW = 2 * NJT * HC
    aux = nc.dram_tensor("aux", [PJ, AUXW], bf16, kind="ExternalInput").ap()
    e1d = nc.dram_tensor("e1d", [PJ, N], fp8, kind="ExternalInput").ap()
    # tiny head: stationaries for the two first-needed tiles (t=15,14)
    auxh = nc.dram_tensor("auxh", [PJ, 2 * HC], bf16, kind="ExternalInput").ap()
    f2p = nc.dram_tensor("f2p", [PJ, NJT], f32, kind="ExternalInput").ap()
    out = nc.dram_tensor("out", [D1, 2 * N], bf16, kind="ExternalOutput").ap()

    g_first = {}
    g_last = {}
    for t in range(NJT - 1, -1, -1):
        for k in range((int(Bt[t]) + BANK - 1) // BANK):
            if k not in g_first:
                g_first[k] = t
            g_last[k] = t

    with ExitStack() as ctx:
        tc = ctx.enter_context(tile.TileContext(nc))
        const = ctx.enter_context(tc.tile_pool(name="const", bufs=1))

        # issue order: keep the early in-flight set minimal -- DMA engines
        # share packet bandwidth fairly across ALL outstanding transfers,
        # so first-needed data must not race the mask bulk
        auxH_sb = const.tile([PJ, 2 * HC], bf16, tag="auxH")
        nc.sync.dma_start(auxH_sb[:], auxh)
        auxA_sb = const.tile([PJ, NJT * HC], bf16, tag="auxA")
        nc.sync.dma_start(auxA_sb[:], aux[:, : NJT * HC])
        f2p_sb = const.tile([PJ, NJT], f32, tag="f2p")
        auxC_sb = const.tile([PJ, N], fp8, tag="auxC")
        nc.sync.dma_start(auxC_sb[:], e1d)
        auxD_sb = const.tile([PJ, NJT * HC], bf16, tag="auxD")

        def stat_he(t):
            if t >= NJT - 2:  # head tile: [t14 | t15]
                o = (t - (NJT - 2)) * HC
                return auxH_sb[:, o : o + D1]
            return auxA_sb[:, t * HC : t * HC + D1]

        def stat_hf(t):
            return auxD_sb[:, t * HC : t * HC + D1]


        GROUPS = [(15, 16), (13, 15), (10, 13), (7, 10), (4, 7), (0, 4)]
        mpool = ctx.enter_context(tc.tile_pool(name="mask", bufs=len(GROUPS)))
        mask_grp = {}
        for gi, (lo, hi) in enumerate(GROUPS):
            mg = mpool.tile([PJ, (hi - lo) * N], fp8, tag="mb")
            if gi == 0:
                # quarter-split the first tile so G-chunk matmuls ungate
                # as each 64KB quarter lands (multi-writer, single tile)
                for qk in range(4):
                    nc.sync.dma_start(
                        mg[:, qk * BANK : (qk + 1) * BANK],
                        maskp[:, lo * N + qk * BANK : lo * N + (qk + 1) * BANK],
                    )
            else:
                nc.sync.dma_start(mg[:], maskp[:, lo * N : hi * N])
            for t in range(lo, hi):
                mask_grp[t] = (mg, t - lo)
            if gi == 0:
                nc.sync.dma_start(f2p_sb[:], f2p)
            if gi == 2:
                nc.sync.dma_start(auxD_sb[:], aux[:, NJT * HC : 2 * NJT * HC])

        def msl(t, c0, c1):
            mg, k = mask_grp[t]
            return mg[:, k * N + c0 : k * N + c1]

        pp = ctx.enter_context(tc.tile_pool(name="psum", bufs=1, space="PSUM"))
        ps = pp.tile([D1, 2 * N], f32, tag="gr")

        rpool = ctx.enter_context(tc.tile_pool(name="relu", bufs=3))
        cpool = ctx.enter_context(tc.tile_pool(name="corr", bufs=3))
        dpool = ctx.enter_context(tc.tile_pool(name="drain", bufs=2))

        # zero the R psum half on DVE during the DMA prefix; every R
        # matmul then accumulates with start=False
        nc.vector.memset(ps[:, N : 2 * N], 0.0)

        gsb = dpool.tile([D1, N], bf16, tag="gsb")
        g_drained = set()

        # ---- pass 1 (descending t): G matmuls + band corrections ----
        for t in range(NJT - 1, -1, -1):
            b0, e0 = int(Bt[t]), int(Et[t])
            w = e0 - b0
            if w > 0:
                # full p on the band: q = max(e1*e2, f2); cm = m*q -> R
                # (stationary: raw hcat); G covers only the pure [0, b0)
                r_sb = rpool.tile([PJ, w], bf16, tag="r")
                nc.vector.tensor_scalar(
                    r_sb[:], auxC_sb[:, b0:e0],
                    f2p_sb[:, t : t + 1], None,
                    op0=ALU.max,
                )
                cm_sb = cpool.tile([PJ, w], bf16, tag="cm")
                nc.vector.tensor_tensor(
                    cm_sb[:], msl(t, b0, e0), r_sb[:], op=ALU.mult
                )
            c = 0
            while c < b0:
                c1 = min(c + BANK, b0)
                k = c // BANK
                nc.tensor.matmul(
                    ps[:, c:c1], stat_he(t), msl(t, c, c1),
                    start=(g_first[k] == t), stop=(g_last[k] == t),
                    skip_group_check=True,
                )
                c = c1
            if w > 0:
                c = b0
                while c < e0:
                    c1 = min((c // BANK + 1) * BANK, e0)
                    nc.tensor.matmul(
                        ps[:, N + c : N + c1], stat_he(t),
                        cm_sb[:, c - b0 : c1 - b0],
                        start=False, stop=False, skip_group_check=True,
                    )
                    c = c1
            for k in range(4):
                if g_last.get(k) == t:
                    sl = slice(k * BANK, (k + 1) * BANK)
                    if k % 2 == 0:
                        nc.scalar.copy(gsb[:, sl], ps[:, sl])
                    else:
                        nc.vector.tensor_copy(gsb[:, sl], ps[:, sl])
                    g_drained.add(k)

        for k in range(4):
            if k not in g_drained:
                nc.vector.memset(gsb[:, k * BANK : (k + 1) * BANK], 0.0)
        nc.sync.dma_start(out[:, :N], gsb[:])

        # ---- pass 2 (ascending t): F matmuls, drain R banks inline ----
        rsb = dpool.tile([D1, N], bf16, tag="rsb")
        for t in range(NJT):
            e0 = int(Et[t])
            c = e0
            while c < N:
                c1 = min((c // BANK + 1) * BANK, N)
                last = (t == NJT - 1) or (int(Et[t + 1]) >= c1)
                nc.tensor.matmul(
                    ps[:, N + c : N + c1], stat_hf(t), msl(t, c, c1),
                    start=False, stop=last, skip_group_check=True,
                )
                if last:
                    k = c // BANK
                    sl = slice(k * BANK, (k + 1) * BANK)
                    if k % 2 == 0:
                        nc.scalar.copy(rsb[:, sl], ps[:, N + k * BANK : N + (k + 1) * BANK])
                    else:
                        nc.vector.tensor_copy(rsb[:, sl], ps[:, N + k * BANK : N + (k + 1) * BANK])
                c = c1
        nc.sync.dma_start(out[:, N:], rsb[:])

    nc.compile()
    return nc


def _prep(x, adj, W, a):
    bf = ml_dtypes.bfloat16
    f8 = ml_dtypes.float8_e4m3fn
    x = np.asarray(x, dtype=np.float32)
    W = np.ascontiguousarray(np.asarray(W, dtype=np.float32))
    a = np.asarray(a, dtype=np.float32)

    h = x @ W
    s1 = h @ a[:DOUT]
    s2 = h @ a[DOUT:]

    pj = [np.argsort(s2[b], kind="stable") for b in range(B)]
    pi = [np.argsort(-s1[b], kind="stable") for b in range(B)]

    t_arr = np.zeros((B, N), np.int64)
    for b in range(B):
        s2s = s2[b][pj[b]]
        t_arr[b] = np.searchsorted(s2s, -s1[b][pi[b]], side="left") // PJ

    Bt = np.zeros(NJT, np.int64)
    Et = np.zeros(NJT, np.int64)
    for t in range(NJT):
        Bt[t] = min(np.searchsorted(t_arr[b], t, side="left") for b in range(B))
        Et[t] = max(np.searchsorted(t_arr[b], t, side="right") for b in range(B))

    adjb = np.asarray(adj) > 0
    in_maps = []
    for b in range(B):
        m = adjb.T[np.ix_(pj[b], pi[b])]
        maskp = np.ascontiguousarray(
            m.reshape(NJT, PJ, N).transpose(1, 0, 2).reshape(PJ, NJT * N)
        ).astype(f8)

        hs = h[b][pj[b]].astype(np.float32)
        e2 = np.exp(s2[b][pj[b]]).astype(np.float32)
        f2 = np.exp(0.2 * s2[b][pj[b]]).astype(np.float32)
        f2pv = np.exp(-0.8 * s2[b][pj[b]]).astype(np.float32)

        he = np.zeros((N, HC), dtype=bf)
        he[:, :DOUT] = (hs * e2[:, None]).astype(bf)
        he[:, DOUT] = e2.astype(bf)
        hf = np.zeros((N, HC), dtype=bf)
        hf[:, :DOUT] = (hs * f2[:, None]).astype(bf)
        hf[:, DOUT] = f2.astype(bf)

        def tile_p(arr):
            return arr.reshape(NJT, PJ, HC).transpose(1, 0, 2).reshape(PJ, NJT * HC)

        e1 = np.exp(0.8 * s1[b][pi[b]]).astype(f8)
        e1d = np.ascontiguousarray(np.broadcast_to(e1[None, :], (PJ, N)))
        aux = np.concatenate([tile_p(he), tile_p(hf)], axis=1)
        f2p_t = np.ascontiguousarray(f2pv.reshape(NJT, PJ).T)

        auxh = np.ascontiguousarray(
            aux[:, (NJT - 2) * HC : NJT * HC])
        in_maps.append({
            "maskp": maskp, "aux": np.ascontiguousarray(aux),
            "f2p": f2p_t, "e1d": e1d, "auxh": auxh,
        })
    return in_maps, (tuple(Bt), tuple(Et)), (pi, s1)


def kernel(x, adj, W, a):
    global LAST_RESULT
    from concourse import bass_utils

    x = np.asarray(x)
    adj = np.asarray(adj)
    assert x.shape == (B, N, DIN) and adj.shape == (N, N)

    in_maps, key, (pi, s1) = _prep(x, adj, W, a)
    if key not in _CACHE:
        _CACHE[key] = _build_nc(np.array(key[0]), np.array(key[1]))
    nc = _CACHE[key]

    res = bass_utils.run_bass_kernel_spmd(
        nc, in_maps, core_ids=list(range(NCORES)),
        trace=bool(int(os.environ.get("GAT_TRACE", "0"))),
    )
    LAST_RESULT = res

    Bt15 = key[0][NJT - 1]
    out = np.empty((B, N, DOUT), dtype=np.float32)
    for b in range(B):
        gr = res.results[b]["out"].astype(np.float64)
        G, R = gr[:, :N], gr[:, N:]
        e1 = np.exp(0.8 * s1[b][pi[b]]).astype(np.float64)
        valid = np.arange(N) < Bt15
        num = np.where(valid[None, :], e1[None, :] * G[:DOUT], 0.0) + R[:DOUT]
        den = np.where(valid, e1 * G[DOUT], 0.0) + R[DOUT]
        outs = (num / den[None, :]).T
        out[b][pi[b]] = outs.astype(np.float32)
    return out
